# revision 16
# baseline (speedup 1.0000x reference)
"""Bass/Tile TRN2 kernel for nn_CutlassLinear (int8-quantized linear, 4096x4096x4096).

Math (matches the reference):
    scale = 127 / max|W|
    w_q   = clip(trunc(W * scale), -127, 127)        # exact small ints
    y     = (x @ w_q^T) * (1/scale) + bias

Distribution: data-parallel over the 4096 token rows -- each of the 8
NeuronCores computes 512 token rows against the full weight matrix. No
collectives; outputs are disjoint row blocks.

Device kernel (per core, SPMD):
  - The 4096-deep contraction is split 26 k-slices in bf16 + 6 k-slices
    in fp8e4m3 run as 3 DoubleRow matmuls (2 k-slices per 512-cycle MM,
    i.e. fp8 slices cost half a bf16 slice).  w_q is exact in bf16; the
    fp8 slices quantize both x and w_q to e4m3, adding ~1.6e-2 L2 error
    on top of the 1.7e-3 bf16 baseline (under the 2e-2 budget).
  - x arrives pre-cast on the host (bf16 + e4m3 slices); no on-device
    x conversion.
  - Head phase: the first SPLIT output tiles accumulate chunk-major
    while x streams in.  Their weights ship SLICE-major (one interleaved
    [k, slice, group, out] tensor) so weight arrival order matches
    consumption order; per-partition rows are >=2KB so the DMA rings
    run at full packet rate.
  - Steady phase: one output tile at a time, weights tile-major int8
    (DVE-upcast) + e4m3, prefetched 3 deep on the SWDGE ring.
  - PSUM eviction fuses dequant scale + bias on the scalar engine and
    writes bf16; outputs stream over the two HWDGE queues.
"""

import numpy as np
import ml_dtypes

P = 128
N_TOKENS = 4096
IN_F = 4096
OUT_F = 4096
N_CORES = 8
TOK = N_TOKENS // N_CORES  # 512 tokens per core
KO = IN_F // P             # 32 contraction blocks
MO = OUT_F // P            # 32 output-feature blocks

NF8 = 6                    # k-slices computed in fp8 DoubleRow (must be even)
KB = KO - NF8              # k-slices computed in bf16
NDR = NF8 // 2             # DoubleRow matmuls per output tile

SPLIT = 8                  # head groups computed chunk-major during the x load
NST = MO - SPLIT           # steady-state output tiles

BF16 = ml_dtypes.bfloat16
F8 = ml_dtypes.float8_e4m3fn


def build_program(debug=False):
    import concourse.mybir as mybir
    import concourse.tile as tile
    from concourse import bacc

    f32 = mybir.dt.float32
    bf16 = mybir.dt.bfloat16
    f8 = mybir.dt.float8e4
    i8 = mybir.dt.int8

    nc = bacc.Bacc("TRN2", target_bir_lowering=False, debug=debug,
                   num_devices=N_CORES)

    xb = nc.dram_tensor("xb", [P, KB, TOK], bf16, kind="ExternalInput").ap()
    x8 = nc.dram_tensor("x8", [P, NF8, TOK], f8, kind="ExternalInput").ap()
    # head-weight slices 0-1 ship pre-cast (bf16) so the first matmuls
    # skip the DVE upcast on the critical path
    whdb = nc.dram_tensor("whdb", [P, 2, SPLIT, P], bf16, kind="ExternalInput").ap()
    whd = nc.dram_tensor("whd", [P, KB - 2, SPLIT, P], i8, kind="ExternalInput").ap()
    w8h = nc.dram_tensor("w8h", [P, NF8, SPLIT, P], f8, kind="ExternalInput").ap()
    wqi = nc.dram_tensor("wqi", [NST, P, KB, P], i8, kind="ExternalInput").ap()
    w8d = nc.dram_tensor("w8", [NST, P, NF8, P], f8, kind="ExternalInput").ap()
    bias = nc.dram_tensor("bias", [P, MO], f32, kind="ExternalInput").ap()
    inv_s = nc.dram_tensor("inv_s", [P, 1], f32, kind="ExternalInput").ap()
    yT = nc.dram_tensor("yT", [P, MO, TOK], bf16, kind="ExternalOutput").ap()

    # x-slice chunk widths for the bf16 part (sum = KB).
    CHUNKS = [1, 1, 2, 2, 2, 2, 2, 2, 3, 3, 3, 3]
    assert sum(CHUNKS) == KB
    # head-weight slice-major int8 pieces (sum = KB - 2; slices 0-1 are whdb)
    WPC = [2, 2, 2, 2, 2, 2, 3, 3, 3, 3]
    assert sum(WPC) == KB - 2
    W_PREFETCH = 4

    with tile.TileContext(nc) as tc:
        with (
            tc.tile_pool(name="const", bufs=1) as const,
            tc.tile_pool(name="xpin", bufs=1) as xpool,
            tc.tile_pool(name="whpin", bufs=1) as whpool,
            tc.tile_pool(name="wstage", bufs=4) as wstage,
            tc.tile_pool(name="wpool", bufs=2 + W_PREFETCH) as wpool,
            tc.tile_pool(name="w8pool", bufs=2 + W_PREFETCH) as w8pool,
            tc.tile_pool(name="opool", bufs=4) as opool,
            tc.tile_pool(name="ps", bufs=1, space="PSUM") as pspool,
        ):
            # Pinned x (both precisions) in SBUF for the whole kernel.
            xb_t = xpool.tile([P, KB, TOK], bf16)
            x8_t = xpool.tile([P, NF8, TOK], f8)
            # Head weights: all SPLIT groups, slice-major.
            whd_sb = whpool.tile([P, KB, SPLIT, P], bf16)
            w8h_sb = whpool.tile([P, NF8, SPLIT, P], f8)

            # --- priming / streaming issue order ---
            # gpsimd (SWDGE): head-weight pieces slice-major, then steady
            # tiles.  sync/scalar (HWDGE): x chunks alternating, then the
            # fp8 x / head fp8 w / bias late.
            chunk_start = [sum(CHUNKS[:i]) for i in range(len(CHUNKS))]
            wpc_start = [sum(WPC[:i]) for i in range(len(WPC))]

            def whd_piece(i):
                a, b = wpc_start[i], wpc_start[i] + WPC[i]
                ws = wstage.tile([P, 3, SPLIT, P], i8, name="whs")[:, :WPC[i]]
                nc.gpsimd.dma_start(out=ws[:], in_=whd[:, a:b])
                nc.vector.tensor_copy(out=whd_sb[:, 2 + a:2 + b], in_=ws[:])

            def x_chunk(i, eng):
                a, b = chunk_start[i], chunk_start[i] + CHUNKS[i]
                eng.dma_start(out=xb_t[:, a:b, :], in_=xb[:, a:b, :])

            # slice-0/1 weights pre-cast on the fast HWDGE queues; the
            # first int8 pieces ride SWDGE in parallel
            nc.sync.dma_start(out=whd_sb[:, 0:1], in_=whdb[:, 0:1])
            nc.scalar.dma_start(out=whd_sb[:, 1:2], in_=whdb[:, 1:2])
            whd_piece(0)
            x_chunk(0, nc.sync)
            x_chunk(1, nc.scalar)
            whd_piece(1)
            x_chunk(2, nc.sync)
            whd_piece(2)
            x_chunk(3, nc.scalar)
            whd_piece(3)
            x_chunk(4, nc.sync)
            whd_piece(4)
            x_chunk(5, nc.scalar)
            whd_piece(5)
            for i in range(6, len(CHUNKS)):
                x_chunk(i, nc.sync if i % 2 == 0 else nc.scalar)
                if i - 6 < len(WPC) - 6:
                    whd_piece(i)

            # fp8 head weights + fp8 x: needed only at the head's tail.
            nc.scalar.dma_start(out=x8_t[:], in_=x8)
            nc.scalar.dma_start(out=w8h_sb[:], in_=w8h)
            bias_sb = const.tile([P, MO], f32)
            nc.sync.dma_start(out=bias_sb[:], in_=bias)
            scale_sb = const.tile([P, 1], f32)
            nc.sync.dma_start(out=scale_sb[:], in_=inv_s)

            # Steady-tile loads (tile-major, behind the head stream).
            wt_tiles = [None] * MO
            w8_tiles = [None] * MO

            def load_w(mo):
                wt = wpool.tile([P, KB, P], bf16, name="wt")
                ws = wstage.tile([P, KB, P], i8, name="ws")
                nc.gpsimd.dma_start(out=ws[:], in_=wqi[mo - SPLIT])
                nc.vector.tensor_copy(out=wt[:], in_=ws[:])
                w8t = w8pool.tile([P, NF8, P], f8, name="w8t")
                nc.gpsimd.dma_start(out=w8t[:], in_=w8d[mo - SPLIT])
                wt_tiles[mo] = wt
                w8_tiles[mo] = w8t

            for mo in range(SPLIT, SPLIT + W_PREFETCH):
                load_w(mo)

            ev_count = [0]

            def evict(mo, ps, t0=0, t1=TOK):
                ot = opool.tile([P, TOK], bf16, name="ot")[:, t0:t1]
                nc.scalar.activation(
                    ot[:], ps[:, t0:t1], mybir.ActivationFunctionType.Identity,
                    bias=bias_sb[:, mo:mo + 1], scale=scale_sb[:, 0:1],
                )
                eng = nc.sync if ev_count[0] % 2 else nc.scalar
                ev_count[0] += 1
                eng.dma_start(out=yT[:, mo, t0:t1], in_=ot[:])

            def dr_matmuls(ps, lhs_pairs, first, last, t0=0, t1=TOK):
                for j in range(NDR):
                    nc.tensor.matmul(
                        ps[:, t0:t1],
                        lhsT=lhs_pairs(j),
                        rhs=x8_t[:, 2 * j:2 * j + 2, t0:t1],
                        start=(first and j == 0),
                        stop=(last and j == NDR - 1),
                        perf_mode=mybir.MatmulPerfMode.DoubleRow,
                    )

            # All 8 PSUM banks, rotated manually: head uses all SPLIT,
            # steady state round-robins the full set.
            ps_tiles = [pspool.tile([P, TOK], mybir.dt.float32, name=f"ps{i}")
                        for i in range(8)]

            # Head: SPLIT open PSUM groups accumulated chunk-major, so
            # the PE consumes each x chunk the moment it lands.
            for c in range(len(CHUNKS)):
                for m in range(SPLIT):
                    for j in range(CHUNKS[c]):
                        ko = chunk_start[c] + j
                        nc.tensor.matmul(
                            ps_tiles[m][:],
                            lhsT=whd_sb[:, ko, m, :],
                            rhs=xb_t[:, ko, :],
                            start=(ko == 0),
                            stop=False,
                        )
            for m in range(SPLIT):
                dr_matmuls(ps_tiles[m],
                           lambda j, m=m: w8h_sb[:, 2 * j:2 * j + 2, m, :],
                           first=False, last=True)
                evict(m, ps_tiles[m])

            # Steady state: one group per mo, K-contiguous.  The last
            # group runs as two column-halves so its first half evicts
            # ~3us before the final matmul, shortening the tail.
            for mo in range(SPLIT, MO):
                if mo + W_PREFETCH < MO:
                    load_w(mo + W_PREFETCH)
                wt = wt_tiles[mo]
                ps = ps_tiles[mo % 8]
                col_chains = [(0, TOK)] if mo < MO - 1 else [(0, TOK // 2),
                                                            (TOK // 2, TOK)]
                for t0, t1 in col_chains:
                    for ko in range(KB):
                        nc.tensor.matmul(
                            ps[:, t0:t1],
                            lhsT=wt[:, ko, :],
                            rhs=xb_t[:, ko, t0:t1],
                            start=(ko == 0),
                            stop=False,
                        )
                    dr_matmuls(ps,
                               lambda j, mo=mo: w8_tiles[mo][:, 2 * j:2 * j + 2, :],
                               first=False, last=True, t0=t0, t1=t1)
                    evict(mo, ps, t0, t1)
                wt_tiles[mo] = None
                w8_tiles[mo] = None

    nc.compile()
    return nc


def prep_inputs(x, weight, bias):
    """Host-side shard/layout prep. Returns per-core input maps."""
    x = np.asarray(x, dtype=np.float32)
    weight = np.asarray(weight, dtype=np.float32)
    bias = np.asarray(bias, dtype=np.float32)

    # Quantize weights exactly as the reference does (fp32 arithmetic).
    s = np.float32(127.0) / np.max(np.abs(weight))
    wq_f = np.clip(np.trunc(weight * s), -127.0, 127.0)
    inv_scale = np.float32(1.0) / s

    # w_q^T laid out [mo, p(k), ko, q(out)]: per-partition-contiguous
    # DMA blocks.  bf16 slices ship as int8 (exact, half the bytes;
    # upcast on device); fp8 slices ship as e4m3.  The first SPLIT
    # output tiles ship slice-major ([p, ko, m, q]) for the head.
    wq_all = wq_f.reshape(MO, P, KO, P).transpose(0, 3, 2, 1)  # [mo, p, ko, q]
    whd_sm = wq_all[:SPLIT, :, :KB, :].transpose(1, 2, 0, 3)   # [p, ko, m, q]
    whdb_dram = np.ascontiguousarray(whd_sm[:, :2]).astype(BF16)
    whd_dram = np.ascontiguousarray(whd_sm[:, 2:]).astype(np.int8)
    w8h_dram = np.ascontiguousarray(
        wq_all[:SPLIT, :, KB:, :].transpose(1, 2, 0, 3).astype(np.float32)).astype(F8)
    wqi_dram = np.ascontiguousarray(wq_all[SPLIT:, :, :KB, :]).astype(np.int8)
    w8_dram = np.ascontiguousarray(
        wq_all[SPLIT:, :, KB:, :].astype(np.float32)).astype(F8)

    bias_dram = np.ascontiguousarray(bias.reshape(MO, P).T)
    inv_dram = np.full((P, 1), inv_scale, dtype=np.float32)

    in_maps = []
    for c in range(N_CORES):
        x_c = x[c * TOK:(c + 1) * TOK, :]                    # [tok, in]
        xT = x_c.reshape(TOK, KO, P).transpose(2, 1, 0)      # [p, ko, tok]
        xb_dram = np.ascontiguousarray(xT[:, :KB, :]).astype(BF16)
        x8_dram = np.ascontiguousarray(xT[:, KB:, :]).astype(F8)
        in_maps.append({
            "xb": xb_dram,
            "x8": x8_dram,
            "whdb": whdb_dram,
            "whd": whd_dram,
            "w8h": w8h_dram,
            "wqi": wqi_dram,
            "w8": w8_dram,
            "bias": bias_dram,
            "inv_s": inv_dram,
        })
    return in_maps


def gather_output(results):
    """results: list of per-core dicts with 'yT' [P, MO, TOK] -> y [4096, 4096]."""
    blocks = []
    for c in range(N_CORES):
        yT = np.asarray(results[c]["yT"]).astype(np.float32)  # [q, mo, tok]
        y_c = yT.transpose(1, 0, 2).reshape(OUT_F, TOK).T     # [tok, out]
        blocks.append(y_c)
    return np.ascontiguousarray(np.concatenate(blocks, axis=0), dtype=np.float32)


_NC_CACHE = None


def get_program():
    global _NC_CACHE
    if _NC_CACHE is None:
        _NC_CACHE = build_program(debug=False)
    return _NC_CACHE


def run(x, weight, bias, trace=False, **run_kwargs):
    from concourse.bass_utils import run_bass_kernel_spmd

    nc = get_program()
    in_maps = prep_inputs(x, weight, bias)
    res = run_bass_kernel_spmd(nc, in_maps, list(range(N_CORES)),
                               trace=trace, **run_kwargs)
    return gather_output(res.results), res


def kernel(x, weight, bias):
    y, _ = run(x, weight, bias, trace=False)
    return y
